# revision 7
# baseline (speedup 1.0000x reference)
"""Trainium2 Bass kernel for nn_AddPoolingFusion.

Reference computation (b=16, l1=l2=2048, d1=d2=d3=768):
    y1  = x1 @ W1.T + b1                      # [b, l1, d3]
    y2  = x2 @ W2.T + b2                      # [b, l2, d3]
    out = y1 + mean(y2, axis=1, keepdims=True)

Because the mean over l2 commutes with the linear layer:
    out[b,i,:] = x1[b,i] @ W1.T + c[b]
    c[b]       = (b1 + b2) + mean_j(x2[b,j]) @ W2.T

Strategy: data-parallel over batch, 2 batches per core, no collectives.
Per core the only heavy compute is the x1 matmul (bf16 on TensorE) and
the x2 mean (DVE accumulate + tiny ones-matmul partition reduce); the
kernel is HBM-bandwidth bound.

Host-side prep (layout/dtype only): x1 is pre-transposed per core to
[768, 4096] so the contraction dim lands on SBUF partitions; x1/x2 and
the weights are pre-cast to bf16 (the TensorE matmul runs in bf16 either
way; rel err stays ~3e-3 vs the 2e-2 gate). The output is stored as bf16
and upcast to f32 on the host.
"""

import os
import sys

import numpy as np

# concourse normally comes from the axon site overlay already on sys.path;
# append /opt/trn_rl_repo as a fallback only.
if "/opt/trn_rl_repo" not in sys.path:
    sys.path.append("/opt/trn_rl_repo")

N_CORES = 8
B_PER_CORE = 2
L = 2048
D = 768  # d1 == d2 == d3 == 768
P = 128
NCH = D // P  # 6 contraction chunks
M = B_PER_CORE * L  # 4096 rows per core
TPG = 8  # m-tiles per DMA group
NGRP = (M // P) // TPG  # 4 groups


def build_nc(debug=False, explicit_ldw=False):
    import concourse.bacc as bacc
    import concourse.mybir as mybir
    import concourse.tile as tile

    f32 = mybir.dt.float32
    bf16 = mybir.dt.bfloat16
    add = mybir.AluOpType.add

    nc = bacc.Bacc(None, target_bir_lowering=False, debug=debug)

    x1t = nc.declare_dram_parameter("x1t", [D, M], bf16, isOutput=False)
    x2s = nc.declare_dram_parameter("x2s", [M, D], bf16, isOutput=False)
    w1t = nc.declare_dram_parameter("w1t", [D, D], bf16, isOutput=False)
    w2t = nc.declare_dram_parameter("w2t", [D, D], bf16, isOutput=False)
    bsum = nc.declare_dram_parameter("bsum", [1, D], f32, isOutput=False)
    out = nc.declare_dram_parameter("out", [M, D], bf16, isOutput=True)

    with tile.TileContext(nc) as tc:
        with (
            tc.tile_pool(name="const", bufs=1) as const,
            tc.tile_pool(name="x2p", bufs=2) as x2p,
            tc.tile_pool(name="x1p", bufs=2) as x1p,
            tc.tile_pool(name="yp", bufs=2) as yp,
            tc.tile_pool(name="small", bufs=1) as small,
            tc.tile_pool(name="psY", bufs=2, space="PSUM") as psY,
            tc.tile_pool(name="psX", bufs=1, space="PSUM") as psX,
            tc.tile_pool(name="psC", bufs=1, space="PSUM") as psC,
        ):
            # ---- weights / constants ----
            # DMA ring split: x1 + W1 go on the Sync HWDGE ring; x2 + W2 +
            # stores go on the Scalar HWDGE ring, so the two streams drain
            # concurrently and neither gates the other's FIFO.
            w1sb = const.tile([P, NCH, D], bf16)
            nc.sync.dma_start(w1sb[:], w1t[:].rearrange("(c p) e -> p c e", p=P))
            w2sb = const.tile([P, NCH, D], bf16)
            nc.scalar.dma_start(w2sb[:], w2t[:].rearrange("(c p) e -> p c e", p=P))
            bsum_sb = const.tile([1, D], f32)
            nc.scalar.dma_start(bsum_sb[:], bsum[:])
            ones_sb = const.tile([P, 1], bf16)
            nc.vector.memset(ones_sb[:], 1.0)

            # ---- x2 mean -> per-batch bias vector c[b], replicated ----
            x2_ap = x2s[:].rearrange("(b t p) d -> b p t d", b=B_PER_CORE, p=P)
            c_rep = []
            for b in range(B_PER_CORE):
                acc = small.tile([P, D], bf16, tag=f"acc{b}")
                for h in range(4):
                    st = x2p.tile([P, 4, D], bf16)
                    nc.scalar.dma_start(st[:], x2_ap[b, :, 4 * h : 4 * h + 4, :])
                    for t in range(4):
                        if h == 0 and t == 0:
                            nc.vector.tensor_copy(acc[:], st[:, 0, :])
                        else:
                            nc.vector.tensor_tensor(acc[:], acc[:], st[:, t, :], op=add)
                # partition-major sum: xbt[p, c] = (1/L) sum_j x2[b, j, c*128+p]
                xbt = small.tile([P, NCH], bf16, tag=f"xbt{b}")
                for c in range(NCH):
                    px = psX.tile([P, 1], f32)
                    nc.tensor.matmul(
                        px[:], acc[:, c * P : (c + 1) * P], ones_sb[:],
                        start=True, stop=True,
                    )
                    # fold the 1/L mean scale in via the copy
                    nc.vector.tensor_scalar_mul(xbt[:, c : c + 1], px[:], 1.0 / L)
                # c_lin = xbar2 @ W2.T   (tiny matmul, K=768, M=1, N=768)
                pc = psC.tile([1, D], f32, tag="pc")
                for c in range(NCH):
                    nc.tensor.matmul(
                        pc[:, 0:512], xbt[:, c : c + 1], w2sb[:, c, 0:512],
                        start=(c == 0), stop=(c == NCH - 1),
                    )
                for c in range(NCH):
                    nc.tensor.matmul(
                        pc[:, 512:768], xbt[:, c : c + 1], w2sb[:, c, 512:768],
                        start=(c == 0), stop=(c == NCH - 1),
                    )
                cs = small.tile([1, D], bf16, tag=f"cs{b}")
                nc.vector.tensor_tensor(cs[:], pc[:], bsum_sb[:], op=add)
                cr = small.tile([P, D], bf16, tag=f"cr{b}")
                nc.gpsimd.partition_broadcast(cr[:], cs[:])
                c_rep.append(cr)

            # ---- main matmul: out = x1 @ W1.T + c[b] ----
            x1_ap = x1t[:].rearrange("(c p) m -> p c m", p=P)  # [128, 6, 4096]
            out_ap = out[:].rearrange("(t p) e -> p t e", p=P)  # [128, 32, 768]
            for g in range(NGRP):
                xs = x1p.tile([P, NCH, TPG * P], bf16)
                nc.sync.dma_start(
                    xs[:], x1_ap[:, :, g * TPG * P : (g + 1) * TPG * P]
                )
                ys = yp.tile([P, TPG, D], bf16)
                for t in range(TPG):
                    mt = g * TPG + t
                    b = mt // (L // P)
                    py_ = psY.tile([P, D], f32)
                    xw = xs[:, :, t * P : (t + 1) * P]
                    for c in range(NCH):
                        if explicit_ldw:
                            nc.tensor.ldweights(xw[:, c, :])
                        nc.tensor.matmul(
                            py_[:, 0:512], xw[:, c, :], w1sb[:, c, 0:512],
                            start=(c == 0), stop=(c == NCH - 1),
                        )
                        nc.tensor.matmul(
                            py_[:, 512:768], xw[:, c, :], w1sb[:, c, 512:768],
                            start=(c == 0), stop=(c == NCH - 1),
                        )
                    # fused evac + bias add (c[b] is ready well before the
                    # first psum tile completes, x2 loads are front-loaded)
                    nc.vector.tensor_tensor(ys[:, t, :], py_[:], c_rep[b][:], op=add)
                    # store every 4 m-tiles for a shorter pipeline tail
                    if t % 4 == 3:
                        lo = g * TPG + t - 3
                        nc.scalar.dma_start(
                            out_ap[:, lo : lo + 4, :], ys[:, t - 3 : t + 1, :]
                        )

    return nc


def make_in_maps(x1, x2, W1, b1, W2, b2):
    import ml_dtypes

    bf16 = ml_dtypes.bfloat16
    w1t_h = np.ascontiguousarray(W1.T).astype(bf16)
    w2t_h = np.ascontiguousarray(W2.T).astype(bf16)
    bsum_h = np.ascontiguousarray((b1 + b2).reshape(1, D).astype(np.float32))
    in_maps = []
    for k in range(N_CORES):
        x1_s = x1[k * B_PER_CORE : (k + 1) * B_PER_CORE]  # [2, 2048, 768]
        x2_s = x2[k * B_PER_CORE : (k + 1) * B_PER_CORE]
        # [768, 4096] with col m = b*2048 + i
        x1t_h = np.ascontiguousarray(np.transpose(x1_s, (2, 0, 1)).reshape(D, M))
        x2s_h = np.ascontiguousarray(x2_s.reshape(M, D))
        in_maps.append(
            {
                "x1t": x1t_h.astype(bf16),
                "x2s": x2s_h.astype(bf16),
                "w1t": w1t_h,
                "w2t": w2t_h,
                "bsum": bsum_h,
            }
        )
    return in_maps


def kernel(x1, x2, W1, b1, W2, b2, trace=False, explicit_ldw=False):
    from concourse.bass_utils import run_bass_kernel_spmd

    nc = build_nc(debug=False, explicit_ldw=explicit_ldw)
    nc.finalize()
    in_maps = make_in_maps(x1, x2, W1, b1, W2, b2)
    res = run_bass_kernel_spmd(
        nc, in_maps, core_ids=list(range(N_CORES)), trace=trace
    )
    shards = [
        res.results[k]["out"].astype(np.float32).reshape(B_PER_CORE, L, D)
        for k in range(N_CORES)
    ]
    out = np.concatenate(shards, axis=0)
    if trace:
        kernel.last_result = res
    return out


# revision 9
# speedup vs baseline: 1.0174x; 1.0174x over previous
"""Trainium2 Bass kernel for nn_AddPoolingFusion.

Reference computation (b=16, l1=l2=2048, d1=d2=d3=768):
    y1  = x1 @ W1.T + b1                      # [b, l1, d3]
    y2  = x2 @ W2.T + b2                      # [b, l2, d3]
    out = y1 + mean(y2, axis=1, keepdims=True)

Because the mean over l2 commutes with the linear layer:
    out[b,i,:] = x1[b,i] @ W1.T + c[b]
    c[b]       = (b1 + b2) + mean_j(x2[b,j]) @ W2.T

Strategy: data-parallel over batch, 2 batches per core, no collectives.
Per core the only heavy compute is the x1 matmul (bf16 on TensorE) and
the x2 mean (DVE accumulate + tiny ones-matmul partition reduce); the
kernel is HBM-bandwidth bound.

Host-side prep (layout/dtype only): x1 is pre-transposed per core to
[768, 4096] so the contraction dim lands on SBUF partitions; x1/x2 and
the weights are pre-cast to bf16 (the TensorE matmul runs in bf16 either
way; rel err stays ~3e-3 vs the 2e-2 gate). The output is stored as bf16
and upcast to f32 on the host.
"""

import os
import sys

import numpy as np

# concourse normally comes from the axon site overlay already on sys.path;
# append /opt/trn_rl_repo as a fallback only.
if "/opt/trn_rl_repo" not in sys.path:
    sys.path.append("/opt/trn_rl_repo")

N_CORES = 8
B_PER_CORE = 2
L = 2048
D = 768  # d1 == d2 == d3 == 768
P = 128
NCH = D // P  # 6 contraction chunks
M = B_PER_CORE * L  # 4096 rows per core
TPG = 8  # m-tiles per DMA group
NGRP = (M // P) // TPG  # 4 groups


def build_nc(debug=False, explicit_ldw=False):
    import concourse.bacc as bacc
    import concourse.mybir as mybir
    import concourse.tile as tile

    f32 = mybir.dt.float32
    bf16 = mybir.dt.bfloat16
    add = mybir.AluOpType.add

    nc = bacc.Bacc(None, target_bir_lowering=False, debug=debug)

    x1t = nc.declare_dram_parameter("x1t", [D, M], bf16, isOutput=False)
    x2s = nc.declare_dram_parameter("x2s", [M, D], bf16, isOutput=False)
    w1t = nc.declare_dram_parameter("w1t", [D, D], bf16, isOutput=False)
    w2t = nc.declare_dram_parameter("w2t", [D, D], bf16, isOutput=False)
    bsum = nc.declare_dram_parameter("bsum", [1, D], f32, isOutput=False)
    out = nc.declare_dram_parameter("out", [M, D], bf16, isOutput=True)

    with tile.TileContext(nc) as tc:
        with (
            tc.tile_pool(name="const", bufs=1) as const,
            tc.tile_pool(name="x2p", bufs=2) as x2p,
            tc.tile_pool(name="x1p", bufs=2) as x1p,
            tc.tile_pool(name="yp", bufs=2) as yp,
            tc.tile_pool(name="small", bufs=1) as small,
            tc.tile_pool(name="psY", bufs=2, space="PSUM") as psY,
            tc.tile_pool(name="psX", bufs=1, space="PSUM") as psX,
            tc.tile_pool(name="psC", bufs=1, space="PSUM") as psC,
        ):
            # ---- weights / constants ----
            # DMA ring split: x1 + W1 go on the Sync HWDGE ring; x2 + W2 +
            # stores go on the Scalar HWDGE ring, so the two streams drain
            # concurrently and neither gates the other's FIFO.
            w1sb = const.tile([P, NCH, D], bf16)
            nc.sync.dma_start(w1sb[:], w1t[:].rearrange("(c p) e -> p c e", p=P))
            ones_sb = const.tile([P, 1], bf16)
            nc.vector.memset(ones_sb[:], 1.0)

            # ---- x2 loads + per-batch accumulate (DVE) ----
            x2_ap = x2s[:].rearrange("(b t p) d -> b p t d", b=B_PER_CORE, p=P)
            accs = []
            w2sb = None

            def x2_acc(b):
                with nc.named_scope(f"x2_acc{b}"):
                    acc = small.tile([P, D], bf16, tag=f"acc{b}")
                    for h in range(4):
                        st = x2p.tile([P, 4, D], bf16)
                        nc.scalar.dma_start(st[:], x2_ap[b, :, 4 * h : 4 * h + 4, :])
                        for t in range(4):
                            if h == 0 and t == 0:
                                nc.vector.tensor_copy(acc[:], st[:, 0, :])
                            else:
                                nc.vector.tensor_tensor(
                                    acc[:], acc[:], st[:, t, :], op=add
                                )
                    accs.append(acc)

            # batch-0 x2 ahead of w2 in the Scalar DMA FIFO: c[0] is the
            # latency-critical input; w2 is only needed once acc0 is reduced.
            x2_acc(0)
            w2sb = const.tile([P, NCH, D], bf16)
            nc.scalar.dma_start(w2sb[:], w2t[:].rearrange("(c p) e -> p c e", p=P))
            bsum_sb = const.tile([1, D], f32)
            nc.scalar.dma_start(bsum_sb[:], bsum[:])
            x2_acc(1)

            def c_path(b):
                # partition-major sum: xbt[p, c] = (1/L) sum_j x2[b, j, c*128+p]
                with nc.named_scope(f"c_path{b}"):
                    acc = accs[b]
                    xbt = small.tile([P, NCH], bf16, tag=f"xbt{b}")
                    for c in range(NCH):
                        px = psX.tile([P, 1], f32)
                        nc.tensor.matmul(
                            px[:], acc[:, c * P : (c + 1) * P], ones_sb[:],
                            start=True, stop=True,
                        )
                        # fold the 1/L mean scale in via the copy
                        nc.vector.tensor_scalar_mul(xbt[:, c : c + 1], px[:], 1.0 / L)
                    # c_lin = xbar2 @ W2.T  (tiny matmul, K=768, M=1, N=768)
                    pc = psC.tile([1, D], f32, tag="pc")
                    for c in range(NCH):
                        nc.tensor.matmul(
                            pc[:, 0:512], xbt[:, c : c + 1], w2sb[:, c, 0:512],
                            start=(c == 0), stop=(c == NCH - 1),
                        )
                    for c in range(NCH):
                        nc.tensor.matmul(
                            pc[:, 512:768], xbt[:, c : c + 1], w2sb[:, c, 512:768],
                            start=(c == 0), stop=(c == NCH - 1),
                        )
                    cs = small.tile([1, D], bf16, tag=f"cs{b}")
                    nc.vector.tensor_tensor(cs[:], pc[:], bsum_sb[:], op=add)
                    cr = small.tile([P, D], bf16, tag=f"cr{b}")
                    nc.gpsimd.partition_broadcast(cr[:], cs[:])
                    return cr

            # ---- main matmul: out = x1 @ W1.T + c[b] ----
            x1_ap = x1t[:].rearrange("(c p) m -> p c m", p=P)  # [128, 6, 4096]
            out_ap = out[:].rearrange("(t p) e -> p t e", p=P)  # [128, 32, 768]
            c_rep = [None, None]

            def group(g):
                with nc.named_scope(f"grp{g}"):
                    xs = x1p.tile([P, NCH, TPG * P], bf16)
                    nc.sync.dma_start(
                        xs[:], x1_ap[:, :, g * TPG * P : (g + 1) * TPG * P]
                    )
                    ys = yp.tile([P, TPG, D], bf16)
                    for t in range(TPG):
                        mt = g * TPG + t
                        b = mt // (L // P)
                        py_ = psY.tile([P, D], f32)
                        xw = xs[:, :, t * P : (t + 1) * P]
                        for c in range(NCH):
                            if explicit_ldw:
                                nc.tensor.ldweights(xw[:, c, :])
                            nc.tensor.matmul(
                                py_[:, 0:512], xw[:, c, :], w1sb[:, c, 0:512],
                                start=(c == 0), stop=(c == NCH - 1),
                            )
                            nc.tensor.matmul(
                                py_[:, 512:768], xw[:, c, :], w1sb[:, c, 512:768],
                                start=(c == 0), stop=(c == NCH - 1),
                            )
                        # fused evac + bias add
                        nc.vector.tensor_tensor(
                            ys[:, t, :], py_[:], c_rep[b][:], op=add
                        )
                        # store every 4 m-tiles for a shorter pipeline tail
                        if t % 4 == 3:
                            lo = g * TPG + t - 3
                            nc.scalar.dma_start(
                                out_ap[:, lo : lo + 4, :], ys[:, t - 3 : t + 1, :]
                            )

            # batch-0 c-path first (gates groups 0-1), batch-1 c-path after
            # group 1 so its PE slot lands when its inputs are long ready.
            c_rep[0] = c_path(0)
            group(0)
            group(1)
            c_rep[1] = c_path(1)
            group(2)
            group(3)

    return nc


def make_in_maps(x1, x2, W1, b1, W2, b2):
    import ml_dtypes

    bf16 = ml_dtypes.bfloat16
    w1t_h = np.ascontiguousarray(W1.T).astype(bf16)
    w2t_h = np.ascontiguousarray(W2.T).astype(bf16)
    bsum_h = np.ascontiguousarray((b1 + b2).reshape(1, D).astype(np.float32))
    in_maps = []
    for k in range(N_CORES):
        x1_s = x1[k * B_PER_CORE : (k + 1) * B_PER_CORE]  # [2, 2048, 768]
        x2_s = x2[k * B_PER_CORE : (k + 1) * B_PER_CORE]
        # [768, 4096] with col m = b*2048 + i
        x1t_h = np.ascontiguousarray(np.transpose(x1_s, (2, 0, 1)).reshape(D, M))
        x2s_h = np.ascontiguousarray(x2_s.reshape(M, D))
        in_maps.append(
            {
                "x1t": x1t_h.astype(bf16),
                "x2s": x2s_h.astype(bf16),
                "w1t": w1t_h,
                "w2t": w2t_h,
                "bsum": bsum_h,
            }
        )
    return in_maps


def kernel(x1, x2, W1, b1, W2, b2, trace=False, explicit_ldw=False):
    from concourse.bass_utils import run_bass_kernel_spmd

    nc = build_nc(debug=False, explicit_ldw=explicit_ldw)
    nc.finalize()
    in_maps = make_in_maps(x1, x2, W1, b1, W2, b2)
    res = run_bass_kernel_spmd(
        nc, in_maps, core_ids=list(range(N_CORES)), trace=trace
    )
    shards = [
        res.results[k]["out"].astype(np.float32).reshape(B_PER_CORE, L, D)
        for k in range(N_CORES)
    ]
    out = np.concatenate(shards, axis=0)
    if trace:
        kernel.last_result = res
    return out
